# revision 19
# baseline (speedup 1.0000x reference)
"""AffinityPropagate prediction kernel for Trainium2 (8 NeuronCores).

Data-parallel over batch B=8: each core owns one image [480, 640].

Layout per core: 120 partitions x 4 image rows. Rows padded to 642 cols
(zero col at each end). State kept in fp32 (single in-place buffer, no
halo slots). Each iteration builds bf16 copies fb (and fbs = fb shifted
one element left) -- double-buffered -- with 2 halo row slots refreshed
by SBUF->SBUF DMA.

Per iteration (matches reference math):
  f_new = w_center*f + sum_{8 taps} w_t * shift_t(f)
Center product + final add in fp32 (dominates accuracy); the 8 other
products and their pairwise-tree sum run in bf16 at DVE 2x mode.
Ops are emitted in 2 row-groups so ACT copies + halo DMAs overlap DVE.
"""

import numpy as np
from contextlib import ExitStack

import concourse.bacc as bacc
import concourse.mybir as mybir
import concourse.tile as tile
from concourse import bass_utils
from concourse.bass_interp import get_hw_module

B, CH, H, W = 8, 8, 480, 640
P = 120            # partitions used (each holds ROWS_P rows)
ROWS_P = H // P    # 4
SLOTS = ROWS_P + 2  # fb/fbs: + top/bottom halo row slots
WPAD = W + 2       # col-padded row: [0, img cols at 1..640, 0]
PX = ROWS_P * W    # 2560 compact px per partition

F32 = mybir.dt.float32
FP16 = mybir.dt.float16
AF = mybir.ActivationFunctionType
OP = mybir.AluOpType

# 8 non-center taps in reference slab order: (dr, dc); w8b slab i = TAPS8[i]
TAPS8 = [(t // 3 - 1, t % 3 - 1) for t in range(9) if t != 4]


def _build(times: int):
    nc = bacc.Bacc("TRN2", debug=False, dynamic_dma_scratch_size=2048)
    aff_d = nc.dram_tensor("affinity", [CH, H * W], F32, kind="ExternalInput")
    feat_d = nc.dram_tensor("feature", [H, W], F32, kind="ExternalInput")
    out_d = nc.dram_tensor("out", [H, W], F32, kind="ExternalOutput")

    with tile.TileContext(nc) as tc, ExitStack() as ctx:
        pool = ctx.enter_context(tc.tile_pool(name="main", bufs=1))

        w8b = pool.tile([P, 8, PX], FP16)        # bf16 tap weights
        w4f = pool.tile([P, PX], F32)            # fp32 center weight
        f32s = pool.tile([P, ROWS_P * WPAD], F32)  # fp32 state (in-place)
        fbp = [pool.tile([P, SLOTS * WPAD], FP16, name=f"fb{i}")
               for i in range(2)]
        fbsp = [pool.tile([P, SLOTS * WPAD], FP16, name=f"fbs{i}")
                for i in range(2)]
        # bf16 scratch for products/tree, per row-group
        pr = [[pool.tile([P, 2 * W], FP16, name=f"pr{g}_{i}") for i in range(4)]
              for g in range(2)]
        sums = pool.tile([P, PX], F32)
        suma = pool.tile([P, PX], F32)
        rec = pool.tile([P, PX], F32)
        # scr shares slots with the per-iteration c32 ring
        scr = pool.tile([P, PX], F32, tag="c32", bufs=2)

        def f3(t):
            return t[:, :].rearrange("p (s w) -> p s w", w=WPAD)

        fs3 = f3(f32s)
        fbv = [f3(t) for t in fbp]
        fbsv = [f3(t) for t in fbsp]

        # ---- zero-init padded buffers ----
        nc.vector.memset(f32s[:, :], 0.0)
        for t in fbp + fbsp:
            nc.vector.memset(t[:, :], 0.0)

        # ---- load feature ----
        feat_v = feat_d[:, :].rearrange("(p r) w -> p r w", r=ROWS_P)
        nc.sync.dma_start(fs3[:, :, 1:1 + W], feat_v)

        # ---- load affinity (channel at a time) + weight prep ----
        aff_v = aff_d[:, :].rearrange("c (p x) -> c p x", x=PX)
        for c in range(CH):
            st = pool.tile([P, PX], F32, name=f"stage{c}", tag="stg", bufs=2)
            nc.sync.dma_start(st[:, :], aff_v[c])
            # bf16 copy of channel into weight slab (normalized later)
            nc.scalar.activation(w8b[:, c, :], st[:, :], AF.Copy)
            if c == 0:
                nc.scalar.activation(sums[:, :], st[:, :], AF.Abs)
                nc.vector.tensor_copy(suma[:, :], st[:, :])
            else:
                absdst = rec if c % 2 else w4f
                nc.scalar.activation(absdst[:, :], st[:, :], AF.Abs)
                nc.vector.tensor_add(sums[:, :], sums[:, :], absdst[:, :])
                nc.vector.tensor_add(suma[:, :], suma[:, :], st[:, :])
        nc.vector.reciprocal_approx_accurate(rec[:, :], sums[:, :], scr[:, :])
        # fp16 copy of rec so slab normalization runs in DVE 2x mode
        rec16 = pool.tile([P, PX], FP16)
        nc.scalar.activation(rec16[:, :], rec[:, :], AF.Copy)
        for i in range(8):
            nc.vector.tensor_mul(w8b[:, i, :], w8b[:, i, :], rec16[:, :])
        # center = 1 - suma * rec  (fp32)
        nc.vector.scalar_tensor_tensor(w4f[:, :], suma[:, :], -1.0, rec[:, :],
                                       OP.mult, OP.mult)
        nc.vector.tensor_scalar_add(w4f[:, :], w4f[:, :], 1.0)

        def make_f16_copies(dfb, dfbs, scale, groups=(0, 1)):
            # fp16 copies of f32 state rows, scaled by 2^-k (fp16 range)
            for g in groups:
                r0 = 2 * g
                nc.scalar.activation(dfb[:, 1 + r0:3 + r0, :],
                                     fs3[:, r0:r0 + 2, :], AF.Copy,
                                     scale=scale)
                nc.scalar.activation(dfbs[:, 1 + r0:3 + r0, 0:W + 1],
                                     fs3[:, r0:r0 + 2, 1:WPAD], AF.Copy,
                                     scale=scale)

        def halo_dmas(dfb, dfbs):
            nc.sync.dma_start(dfb[1:P, 0, :], dfb[0:P - 1, ROWS_P, :])
            nc.sync.dma_start(dfb[0:P - 1, SLOTS - 1, :], dfb[1:P, 1, :])
            nc.sync.dma_start(dfbs[1:P, 0, :], dfbs[0:P - 1, ROWS_P, :])
            nc.sync.dma_start(dfbs[0:P - 1, SLOTS - 1, :], dfbs[1:P, 1, :])

        make_f16_copies(fbv[0], fbsv[0], 1.0)
        halo_dmas(fbv[0], fbsv[0])

        wv = w8b[:, :, :].rearrange("p s (r w) -> p s r w", w=W)
        w4v = w4f[:, :].rearrange("p (r w) -> p r w", w=W)

        # ---- iterations ----
        for it in range(times):
            cfb, cfbs = fbv[it % 2], fbsv[it % 2]
            nfb, nfbs = fbv[(it + 1) % 2], fbsv[(it + 1) % 2]
            for g in (0, 1):
                r0 = 2 * g          # f32 row offset of this group
                s0 = 1 + r0         # fb slot offset
                a, b, c_, d = pr[g]

                def mul8(dst, k):
                    dr, dc = TAPS8[k]
                    wvg = wv[:, k, r0:r0 + 2, :]
                    if dc == 0:
                        src = cfbs[:, s0 + dr:s0 + 2 + dr, 0:W]
                    else:
                        src = cfb[:, s0 + dr:s0 + 2 + dr, 1 + dc:1 + dc + W]
                    nc.vector.tensor_mul(
                        dst[:, :].rearrange("p (r w) -> p r w", w=W), src, wvg)

                def add2(dst, x, y):
                    nc.vector.tensor_add(dst[:, :], x[:, :], y[:, :])

                mul8(a, 0); mul8(b, 1); add2(a, a, b)
                mul8(b, 2); mul8(c_, 3); add2(b, b, c_)
                add2(a, a, b)
                mul8(b, 4); mul8(c_, 5); add2(b, b, c_)
                mul8(c_, 6); mul8(d, 7); add2(c_, c_, d)
                add2(b, b, c_)
                add2(a, a, b)
                # center product fp32: c32 = w4f * f32
                c32 = pool.tile([P, 2 * W], F32, name=f"c32_{it}_{g}",
                                tag="c32", bufs=2)
                c32v = c32[:, :].rearrange("p (r w) -> p r w", w=W)
                nc.gpsimd.tensor_mul(c32v, fs3[:, r0:r0 + 2, 1:1 + W],
                                     w4v[:, r0:r0 + 2, :])
                # final: f32 = tree * 2^it + c32 (in place, padded interior)
                nc.vector.scalar_tensor_tensor(
                    fs3[:, r0:r0 + 2, 1:1 + W],
                    a[:, :].rearrange("p (r w) -> p r w", w=W),
                    float(2.0 ** it), c32v, OP.mult, OP.add)
                if it != times - 1:
                    make_f16_copies(nfb, nfbs, float(2.0 ** -(it + 1)),
                                    groups=(g,))
                else:
                    # overlap the store with the other group's compute
                    out_v = out_d[:, :].rearrange("(p r) w -> p r w",
                                                  r=ROWS_P)
                    nc.sync.dma_start(out_v[:, r0:r0 + 2, :],
                                      fs3[:, r0:r0 + 2, 1:1 + W])
            if it != times - 1:
                halo_dmas(nfb, nfbs)

        if times == 0:
            out_v = out_d[:, :].rearrange("(p r) w -> p r w", r=ROWS_P)
            nc.sync.dma_start(out_v, fs3[:, :, 1:1 + W])

    nc.compile()
    nc.m = get_hw_module(nc.m)
    return nc


_CACHE = {}


def _get(times: int):
    if times not in _CACHE:
        _CACHE[times] = _build(times)
    return _CACHE[times]


def kernel(affinity, feature, times, _trace=False, _trace_kwargs=None):
    t = int(times)
    nc = _get(t)
    aff = np.ascontiguousarray(affinity, dtype=np.float32)
    fea = np.ascontiguousarray(feature, dtype=np.float32)
    in_maps = [
        {"affinity": aff[b].reshape(CH, H * W), "feature": fea[b, 0]}
        for b in range(B)
    ]
    res = bass_utils.run_bass_kernel_spmd(
        nc, in_maps, core_ids=list(range(B)),
        trace=_trace, **(_trace_kwargs or {}),
    )
    out = np.stack([res.results[b]["out"] for b in range(B)])[:, None]
    if _trace:
        return out.astype(np.float32), res
    return out.astype(np.float32)
